# revision 11
# baseline (speedup 1.0000x reference)
"""Trainium2 Bass kernel for dynamic int8-quantized linear layer.

Reference computation (per nn_CustomLinear):
  - per-row symmetric int8 quantization of weight [O, D]
  - dynamic per-row symmetric int8 quantization of x [B, N, D]
  - int8 GEMM accumulated in int32
  - dequantize with x_scale (per row) * w_scale (per out channel) + bias

Strategy:
  - Data-parallel over 8 NeuronCores: x flattened to [B*N, D] and split in 8
    row shards; weight + bias replicated on every core.
  - On-device per core: quantize W and x-shard rows to integer values stored
    as bf16 (integers in [-127, 127] are exact in bf16), move the quantized
    tensors through DRAM with X-bar transpose DMAs so the contraction dim (D)
    lands on the partition axis, run the GEMM on the tensor engine in bf16
    with fp32 PSUM accumulation (exact for these magnitudes: products <=
    16129, sums << 2^24), then dequantize (ACT per-partition x_scale, DVE
    multiply by broadcast w_scale, DVE bias add).
  - Rounding matches jnp.round (half-to-even) via the fp32 magic constant
    (1.5 * 2^23) add/subtract trick.
"""

import numpy as np

import concourse.bass as bass
import concourse.mybir as mybir
import concourse.tile as tile
from concourse import bacc
from concourse.bass_utils import run_bass_kernel_spmd

F32 = mybir.dt.float32
BF16 = mybir.dt.bfloat16

RND = 12582912.0  # 1.5 * 2**23: adding then subtracting rounds fp32 to int (RNE)
QMAX = 127.0

# Problem shapes (hardcoded; harness calls kernel() with exactly these).
B, N, D, O = 4, 4096, 2048, 2048
N_CORES = 8
P = 128


def build_nc(n_rows=B * N // N_CORES, d=D, o=O, n_cores=N_CORES):
    """Build the single-core Bass program (SPMD: same program on all cores)."""
    nc = bacc.Bacc(
        "TRN2",
        target_bir_lowering=False,
        debug=False,
        num_devices=n_cores,
    )
    x_d = nc.dram_tensor("x", [n_rows, d], F32, kind="ExternalInput").ap()
    o_sh = o // n_cores  # weight rows quantized on this core
    w_d = nc.dram_tensor("w", [o_sh, d], F32, kind="ExternalInput").ap()
    b_d = nc.dram_tensor("b", [o], F32, kind="ExternalInput").ap()
    y_d = nc.dram_tensor("y", [n_rows, o], F32, kind="ExternalOutput").ap()

    n_nt = n_rows // P  # number of x row tiles
    n_ot = o_sh // P  # number of weight row tiles (per-core shard)
    n_dt = d // P  # number of contraction tiles
    OC = min(512, o)  # matmul free-dim chunk (one PSUM bank)
    n_oc = o // OC
    QC = min(512, d)  # chunk width for the fp32 rounding temp
    n_qc = d // QC
    NCH = min(512, n_rows)  # x rows per transpose chunk
    n_ch = n_rows // NCH
    tpc = NCH // P  # x tiles per chunk

    with tile.TileContext(nc) as tc:
        with (
            tc.tile_pool(name="consts", bufs=1) as consts,
            tc.tile_pool(name="wqt_pool", bufs=1) as wqt_pool,
            tc.tile_pool(name="dram", bufs=1, space="DRAM") as dram,
            tc.tile_pool(name="dramx", bufs=2, space="DRAM") as dramx,
            tc.tile_pool(name="xin", bufs=2) as xin,
            tc.tile_pool(name="xtmp", bufs=2) as xtmp,
            tc.tile_pool(name="dq", bufs=2) as dq,
            tc.tile_pool(name="yout", bufs=2) as yout,
            tc.tile_pool(name="psum_mm", bufs=2, space="PSUM") as psum_mm_pool,
        ):
            rnd_c = consts.tile([P, 1], F32)
            nc.vector.memset(rnd_c, RND)
            nrnd_c = consts.tile([P, 1], F32)
            nc.vector.memset(nrnd_c, -RND)

            # per-row stat slabs (partition = row within tile, column = tile idx)
            wmax_slab = consts.tile([P, n_ot], F32)
            ws_slab = consts.tile([P, n_ot], F32)
            wrecip_slab = consts.tile([P, n_ot], F32)
            xmax_slab = consts.tile([P, n_nt], F32)
            xs_slab = consts.tile([P, n_nt], F32)
            xrecip_slab = consts.tile([P, n_nt], F32)

            # w_scale broadcast to all partitions, and bias broadcast
            wsb = consts.tile([P, o], F32)
            biasb = consts.tile([P, o], F32)

            # bias broadcast: DRAM [o] -> SBUF [P, o] with 0-stride partition
            nc.gpsimd.dma_start(
                out=biasb,
                in_=bass.AP(
                    tensor=b_d.tensor, offset=b_d.offset, ap=[[0, P]] + list(b_d.ap)
                ),
            )

            # quantized transposed weight, resident in SBUF: [d_part, d_tile, o]
            wqt = wqt_pool.tile([P, n_dt, o], BF16)

            # ---------------- W phase (sharded across cores) ----------------
            cc_in = dram.tile([o_sh, d], BF16, name="cc_in")
            with (
                tc.tile_pool(name="win", bufs=2) as win,
                tc.tile_pool(name="wtmp", bufs=2) as wtmp,
            ):
                for t in range(n_ot):
                    w_t = win.tile([P, d], F32, name="w_t")
                    nc.gpsimd.dma_start(out=w_t, in_=w_d[t * P : (t + 1) * P, :])
                    nc.vector.tensor_reduce(
                        out=wmax_slab[:, t : t + 1],
                        in_=w_t,
                        axis=mybir.AxisListType.X,
                        op=mybir.AluOpType.max,
                        apply_absolute_value=True,
                    )
                    # w_scale = clip(wmax, 1e-8, inf) / 127
                    nc.vector.tensor_scalar(
                        out=ws_slab[:, t : t + 1],
                        in0=wmax_slab[:, t : t + 1],
                        scalar1=1e-8,
                        scalar2=1.0 / QMAX,
                        op0=mybir.AluOpType.max,
                        op1=mybir.AluOpType.mult,
                    )
                    nc.vector.reciprocal(
                        out=wrecip_slab[:, t : t + 1], in_=ws_slab[:, t : t + 1]
                    )
                    # wq = round(w / w_scale)  (RNE, matches jnp.round)
                    wq = wtmp.tile([P, d], BF16, name="wq")
                    for c in range(n_qc):
                        tw = wtmp.tile([P, QC], F32, name="tw")
                        nc.scalar.activation(
                            out=tw,
                            in_=w_t[:, c * QC : (c + 1) * QC],
                            func=mybir.ActivationFunctionType.Identity,
                            bias=rnd_c,
                            scale=wrecip_slab[:, t : t + 1],
                        )
                        nc.vector.tensor_scalar_add(
                            wq[:, c * QC : (c + 1) * QC], tw, -RND
                        )
                    nc.sync.dma_start(
                        out=cc_in[t * P : (t + 1) * P, :], in_=wq
                    )

                # gather the full quantized weight [o, d] across cores
                if n_cores > 1:
                    cc_out = dram.tile(
                        [n_cores * o_sh, d], BF16, name="cc_out",
                        addr_space="Shared",
                    )
                    nc.gpsimd.collective_compute(
                        "AllGather",
                        mybir.AluOpType.bypass,
                        replica_groups=[list(range(n_cores))],
                        ins=[cc_in.opt()],
                        outs=[cc_out.opt()],
                    )
                    wq_full = cc_out
                else:
                    wq_full = cc_in

                # w_scale row: scatter shard slab -> DRAM, gather, broadcast
                ws_sh = dram.tile([n_ot, P], F32, name="ws_sh")
                nc.gpsimd.dma_start(out=ws_sh.rearrange("t p -> p t"), in_=ws_slab)
                if n_cores > 1:
                    ws_out = dram.tile(
                        [n_cores, n_ot * P], F32, name="ws_out",
                        addr_space="Shared",
                    )
                    nc.gpsimd.collective_compute(
                        "AllGather",
                        mybir.AluOpType.bypass,
                        replica_groups=[list(range(n_cores))],
                        ins=[ws_sh.rearrange("t p -> (t p)").opt()],
                        outs=[ws_out.opt()],
                    )
                    ws_flat = ws_out.rearrange("c s -> (c s)")
                else:
                    ws_flat = ws_sh.rearrange("t p -> (t p)")
                nc.gpsimd.dma_start(
                    out=wsb,
                    in_=bass.AP(
                        tensor=ws_flat.tensor,
                        offset=ws_flat.offset,
                        ap=[[0, P]] + list(ws_flat.ap),
                    ),
                )

            # ---------------- X phase ----------------
            # Prep (quantize+store+transpose) up to 3 chunks BEFORE the
            # weight transpose-loads hit the sync ring: the W transposes
            # wait on the AllGather, and anything queued behind them on the
            # HWDGE ring would head-of-line block.
            with tc.tile_pool(name="xqtc_pool", bufs=3) as xqtc_pool:
                xqtcs = {}

                def prep_chunk(ch):
                    xq_dram = dramx.tile([NCH, d], BF16, name="xq_dram")
                    for j in range(tpc):
                        i = ch * tpc + j
                        x_t = xin.tile([P, d], F32, name="x_t")
                        nc.gpsimd.dma_start(
                            out=x_t, in_=x_d[i * P : (i + 1) * P, :]
                        )
                        nc.vector.tensor_reduce(
                            out=xmax_slab[:, i : i + 1],
                            in_=x_t,
                            axis=mybir.AxisListType.X,
                            op=mybir.AluOpType.max,
                            apply_absolute_value=True,
                        )
                        # x_scale = max(xmax / 127, 1e-12)
                        nc.vector.tensor_scalar(
                            out=xs_slab[:, i : i + 1],
                            in0=xmax_slab[:, i : i + 1],
                            scalar1=1.0 / QMAX,
                            scalar2=1e-12,
                            op0=mybir.AluOpType.mult,
                            op1=mybir.AluOpType.max,
                        )
                        nc.vector.reciprocal(
                            out=xrecip_slab[:, i : i + 1], in_=xs_slab[:, i : i + 1]
                        )
                        xq = xtmp.tile([P, d], BF16, name="xq")
                        for c in range(n_qc):
                            tx = xtmp.tile([P, QC], F32, name="tx")
                            nc.scalar.activation(
                                out=tx,
                                in_=x_t[:, c * QC : (c + 1) * QC],
                                func=mybir.ActivationFunctionType.Identity,
                                bias=rnd_c,
                                scale=xrecip_slab[:, i : i + 1],
                            )
                            nc.vector.tensor_scalar_add(
                                xq[:, c * QC : (c + 1) * QC], tx, -RND
                            )
                        nc.scalar.dma_start(
                            out=xq_dram[j * P : (j + 1) * P, :], in_=xq
                        )
                    # transpose-load the chunk: [NCH, d] -> [d, NCH]
                    xqtc = xqtc_pool.tile([P, n_dt, NCH], BF16, name="xqtc")
                    for dd in range(n_dt):
                        nc.sync.dma_start_transpose(
                            out=xqtc[:, dd, :],
                            in_=xq_dram[:, dd * P : (dd + 1) * P],
                        )
                    xqtcs[ch] = xqtc

                def gemm_chunk(ch):
                    xqtc = xqtcs.pop(ch)
                    for j in range(tpc):
                        i = ch * tpc + j
                        y_t = yout.tile([P, o], F32, name="y_t")
                        pms = [
                            psum_mm_pool.tile([P, OC], F32, name=f"pm{oc}")
                            for oc in range(n_oc)
                        ]
                        for dd in range(n_dt):
                            for oc in range(n_oc):
                                nc.tensor.matmul(
                                    pms[oc],
                                    lhsT=xqtc[:, dd, j * P : (j + 1) * P],
                                    rhs=wqt[:, dd, oc * OC : (oc + 1) * OC],
                                    start=(dd == 0),
                                    stop=(dd == n_dt - 1),
                                )
                        for oc in range(n_oc):
                            # dequant: y = (acc * x_scale) * w_scale + bias
                            t1 = dq.tile([P, OC], F32, name="t1")
                            nc.scalar.activation(
                                out=t1,
                                in_=pms[oc],
                                func=mybir.ActivationFunctionType.Identity,
                                bias=0.0,
                                scale=xs_slab[:, i : i + 1],
                            )
                            t2 = dq.tile([P, OC], F32, name="t2")
                            nc.vector.tensor_mul(
                                t2, t1, wsb[:, oc * OC : (oc + 1) * OC]
                            )
                            nc.vector.tensor_add(
                                y_t[:, oc * OC : (oc + 1) * OC],
                                t2,
                                biasb[:, oc * OC : (oc + 1) * OC],
                            )
                        nc.scalar.dma_start(
                            out=y_d[i * P : (i + 1) * P, :], in_=y_t
                        )

                n_pre = min(3, n_ch)
                for ch in range(n_pre):
                    prep_chunk(ch)

                # W transposes go on the sync ring only now (they wait on
                # the AllGather); X chunk transposes for later chunks queue
                # behind them, which is fine (the GEMM needs wqt anyway).
                for dd in range(n_dt):
                    nc.sync.dma_start_transpose(
                        out=wqt[:, dd, :], in_=wq_full[:, dd * P : (dd + 1) * P]
                    )

                for ch in range(n_ch):
                    gemm_chunk(ch)
                    if ch + n_pre < n_ch:
                        prep_chunk(ch + n_pre)

    nc.compile()
    return nc


_NC_CACHE = {}


def _get_nc(n_rows, d, o, n_cores):
    key = (n_rows, d, o, n_cores)
    if key not in _NC_CACHE:
        _NC_CACHE[key] = build_nc(n_rows, d, o, n_cores)
    return _NC_CACHE[key]


def kernel(x: np.ndarray, weight: np.ndarray, bias: np.ndarray, **run_kwargs):
    b, n, d = x.shape
    o = weight.shape[0]
    rows = b * n
    n_rows = rows // N_CORES
    nc = _get_nc(n_rows, d, o, N_CORES)

    x_flat = np.ascontiguousarray(np.asarray(x, dtype=np.float32).reshape(rows, d))
    w = np.ascontiguousarray(np.asarray(weight, dtype=np.float32))
    bias = np.ascontiguousarray(np.asarray(bias, dtype=np.float32))

    o_sh = o // N_CORES
    in_maps = [
        {
            "x": x_flat[c * n_rows : (c + 1) * n_rows],
            "w": np.ascontiguousarray(w[c * o_sh : (c + 1) * o_sh]),
            "b": bias,
        }
        for c in range(N_CORES)
    ]
    res = run_bass_kernel_spmd(nc, in_maps, list(range(N_CORES)), **run_kwargs)
    y = np.concatenate([res.results[c]["y"] for c in range(N_CORES)], axis=0)
    out = y.reshape(b, n, o).astype(x.dtype, copy=False)
    if run_kwargs:
        return out, res
    return out


if __name__ == "__main__":
    x = np.random.randn(B, N, D).astype(np.float32)
    w = np.random.randn(O, D).astype(np.float32)
    bias = np.random.randn(O).astype(np.float32)
    y = kernel(x, w, bias)
    print(y.shape, y.dtype)


# revision 12
# speedup vs baseline: 1.1392x; 1.1392x over previous
"""Trainium2 Bass kernel for dynamic int8-quantized linear layer.

Reference computation (per nn_CustomLinear):
  - per-row symmetric int8 quantization of weight [O, D]
  - dynamic per-row symmetric int8 quantization of x [B, N, D]
  - int8 GEMM accumulated in int32
  - dequantize with x_scale (per row) * w_scale (per out channel) + bias

Strategy:
  - Data-parallel over 8 NeuronCores: x flattened to [B*N, D] and split in 8
    row shards; weight + bias replicated on every core.
  - On-device per core: quantize W and x-shard rows to integer values stored
    as bf16 (integers in [-127, 127] are exact in bf16), move the quantized
    tensors through DRAM with X-bar transpose DMAs so the contraction dim (D)
    lands on the partition axis, run the GEMM on the tensor engine in bf16
    with fp32 PSUM accumulation (exact for these magnitudes: products <=
    16129, sums << 2^24), then dequantize (ACT per-partition x_scale, DVE
    multiply by broadcast w_scale, DVE bias add).
  - Rounding matches jnp.round (half-to-even) via the fp32 magic constant
    (1.5 * 2^23) add/subtract trick.
"""

import numpy as np

import concourse.bass as bass
import concourse.mybir as mybir
import concourse.tile as tile
from concourse import bacc
from concourse.bass_utils import run_bass_kernel_spmd

F32 = mybir.dt.float32
BF16 = mybir.dt.bfloat16

RND = 12582912.0  # 1.5 * 2**23: adding then subtracting rounds fp32 to int (RNE)
QMAX = 127.0

# Problem shapes (hardcoded; harness calls kernel() with exactly these).
B, N, D, O = 4, 4096, 2048, 2048
N_CORES = 8
P = 128


def build_nc(n_rows=B * N // N_CORES, d=D, o=O, n_cores=N_CORES):
    """Build the single-core Bass program (SPMD: same program on all cores)."""
    nc = bacc.Bacc(
        "TRN2",
        target_bir_lowering=False,
        debug=False,
        num_devices=n_cores,
    )
    x_d = nc.dram_tensor("x", [n_rows, d], F32, kind="ExternalInput").ap()
    w_d = nc.dram_tensor("w", [o, d], F32, kind="ExternalInput").ap()
    b_d = nc.dram_tensor("b", [o], F32, kind="ExternalInput").ap()
    y_d = nc.dram_tensor("y", [n_rows, o], F32, kind="ExternalOutput").ap()

    n_nt = n_rows // P  # number of x row tiles
    n_ot = o // P  # number of weight row tiles
    n_dt = d // P  # number of contraction tiles
    OC = min(512, o)  # matmul free-dim chunk (one PSUM bank)
    n_oc = o // OC
    QC = min(512, d)  # chunk width for the fp32 rounding temp
    n_qc = d // QC
    NCH = min(512, n_rows)  # x rows per transpose chunk
    n_ch = n_rows // NCH
    tpc = NCH // P  # x tiles per chunk

    with tile.TileContext(nc) as tc:
        with (
            tc.tile_pool(name="consts", bufs=1) as consts,
            tc.tile_pool(name="wqt_pool", bufs=1) as wqt_pool,
            tc.tile_pool(name="dram", bufs=1, space="DRAM") as dram,
            tc.tile_pool(name="dramx", bufs=2, space="DRAM") as dramx,
            tc.tile_pool(name="xin", bufs=2) as xin,
            tc.tile_pool(name="xtmp", bufs=3) as xtmp,
            tc.tile_pool(name="dq", bufs=2) as dq,
            tc.tile_pool(name="yout", bufs=2) as yout,
            tc.tile_pool(name="psum_mm", bufs=4, space="PSUM") as psum_mm_pool,
        ):
            rnd_c = consts.tile([P, 1], F32)
            nc.vector.memset(rnd_c, RND)
            nrnd_c = consts.tile([P, 1], F32)
            nc.vector.memset(nrnd_c, -RND)

            # per-row stat slabs (partition = row within tile, column = tile idx)
            wmax_slab = consts.tile([P, n_ot], F32)
            ws_slab = consts.tile([P, n_ot], F32)
            wrecip_slab = consts.tile([P, n_ot], F32)
            xmax_slab = consts.tile([P, n_nt], F32)
            xs_slab = consts.tile([P, n_nt], F32)
            xrecip_slab = consts.tile([P, n_nt], F32)

            # w_scale broadcast to all partitions, and bias broadcast
            wsb = consts.tile([P, o], F32)
            biasb = consts.tile([P, o], F32)

            # bias broadcast: DRAM [o] -> SBUF [P, o] with 0-stride partition
            nc.gpsimd.dma_start(
                out=biasb,
                in_=bass.AP(
                    tensor=b_d.tensor, offset=b_d.offset, ap=[[0, P]] + list(b_d.ap)
                ),
            )

            # quantized transposed weight, resident in SBUF: [d_part, d_tile, o]
            wqt = wqt_pool.tile([P, n_dt, o], BF16)

            # ---------------- W phase ----------------
            wq_dram = dram.tile([o, d], BF16)
            with (
                tc.tile_pool(name="win", bufs=2) as win,
                tc.tile_pool(name="wtmp", bufs=2) as wtmp,
            ):
                for t in range(n_ot):
                    w_t = win.tile([P, d], F32, name="w_t")
                    nc.gpsimd.dma_start(out=w_t, in_=w_d[t * P : (t + 1) * P, :])
                    nc.vector.tensor_reduce(
                        out=wmax_slab[:, t : t + 1],
                        in_=w_t,
                        axis=mybir.AxisListType.X,
                        op=mybir.AluOpType.max,
                        apply_absolute_value=True,
                    )
                    # w_scale = clip(wmax, 1e-8, inf) / 127
                    nc.vector.tensor_scalar(
                        out=ws_slab[:, t : t + 1],
                        in0=wmax_slab[:, t : t + 1],
                        scalar1=1e-8,
                        scalar2=1.0 / QMAX,
                        op0=mybir.AluOpType.max,
                        op1=mybir.AluOpType.mult,
                    )
                    nc.vector.reciprocal(
                        out=wrecip_slab[:, t : t + 1], in_=ws_slab[:, t : t + 1]
                    )
                    # wq = round(w / w_scale)  (RNE, matches jnp.round)
                    wq = wtmp.tile([P, d], BF16, name="wq")
                    for c in range(n_qc):
                        tw = wtmp.tile([P, QC], F32, name="tw")
                        nc.scalar.activation(
                            out=tw,
                            in_=w_t[:, c * QC : (c + 1) * QC],
                            func=mybir.ActivationFunctionType.Identity,
                            bias=rnd_c,
                            scale=wrecip_slab[:, t : t + 1],
                        )
                        nc.vector.tensor_scalar_add(
                            wq[:, c * QC : (c + 1) * QC], tw, -RND
                        )
                    nc.sync.dma_start(out=wq_dram[t * P : (t + 1) * P, :], in_=wq)

                # transpose-load the full quantized weight: [o, d] -> [d, o]
                for dd in range(n_dt):
                    nc.sync.dma_start_transpose(
                        out=wqt[:, dd, :], in_=wq_dram[:, dd * P : (dd + 1) * P]
                    )

                # w_scale row: scatter slab -> DRAM flat [o], broadcast back
                ws_dram = dram.tile([n_ot, P], F32)
                nc.gpsimd.dma_start(out=ws_dram.rearrange("t p -> p t"), in_=ws_slab)
                ws_flat = ws_dram.rearrange("t p -> (t p)")
                nc.gpsimd.dma_start(
                    out=wsb,
                    in_=bass.AP(
                        tensor=ws_flat.tensor,
                        offset=ws_flat.offset,
                        ap=[[0, P]] + list(ws_flat.ap),
                    ),
                )

            # ---------------- X phase ----------------
            with tc.tile_pool(name="xqtc_pool", bufs=2) as xqtc_pool:
                for ch in range(n_ch):
                    xq_dram = dramx.tile([NCH, d], BF16, name="xq_dram")
                    for j in range(tpc):
                        i = ch * tpc + j
                        x_t = xin.tile([P, d], F32, name="x_t")
                        nc.gpsimd.dma_start(out=x_t, in_=x_d[i * P : (i + 1) * P, :])
                        nc.vector.tensor_reduce(
                            out=xmax_slab[:, i : i + 1],
                            in_=x_t,
                            axis=mybir.AxisListType.X,
                            op=mybir.AluOpType.max,
                            apply_absolute_value=True,
                        )
                        # x_scale = max(xmax / 127, 1e-12)
                        nc.vector.tensor_scalar(
                            out=xs_slab[:, i : i + 1],
                            in0=xmax_slab[:, i : i + 1],
                            scalar1=1.0 / QMAX,
                            scalar2=1e-12,
                            op0=mybir.AluOpType.mult,
                            op1=mybir.AluOpType.max,
                        )
                        nc.vector.reciprocal(
                            out=xrecip_slab[:, i : i + 1], in_=xs_slab[:, i : i + 1]
                        )
                        xq = xtmp.tile([P, d], BF16, name="xq")
                        for c in range(n_qc):
                            tx = xtmp.tile([P, QC], F32, name="tx")
                            nc.scalar.activation(
                                out=tx,
                                in_=x_t[:, c * QC : (c + 1) * QC],
                                func=mybir.ActivationFunctionType.Identity,
                                bias=rnd_c,
                                scale=xrecip_slab[:, i : i + 1],
                            )
                            nc.vector.tensor_scalar_add(
                                xq[:, c * QC : (c + 1) * QC], tx, -RND
                            )
                        nc.sync.dma_start(
                            out=xq_dram[j * P : (j + 1) * P, :], in_=xq
                        )

                    # transpose-load the chunk: [NCH, d] -> [d, NCH]
                    xqtc = xqtc_pool.tile([P, n_dt, NCH], BF16, name="xqtc")
                    for dd in range(n_dt):
                        nc.sync.dma_start_transpose(
                            out=xqtc[:, dd, :],
                            in_=xq_dram[:, dd * P : (dd + 1) * P],
                        )

                    for j in range(tpc):
                        i = ch * tpc + j
                        y_t = yout.tile([P, o], F32, name="y_t")
                        for oc in range(n_oc):
                            pm = psum_mm_pool.tile([P, OC], F32, name="pm")
                            for dd in range(n_dt):
                                nc.tensor.matmul(
                                    pm,
                                    lhsT=xqtc[:, dd, j * P : (j + 1) * P],
                                    rhs=wqt[:, dd, oc * OC : (oc + 1) * OC],
                                    start=(dd == 0),
                                    stop=(dd == n_dt - 1),
                                )
                            # dequant: y = (acc * x_scale) * w_scale + bias
                            t1 = dq.tile([P, OC], F32, name="t1")
                            nc.scalar.activation(
                                out=t1,
                                in_=pm,
                                func=mybir.ActivationFunctionType.Identity,
                                bias=0.0,
                                scale=xs_slab[:, i : i + 1],
                            )
                            t2 = dq.tile([P, OC], F32, name="t2")
                            nc.vector.tensor_mul(
                                t2, t1, wsb[:, oc * OC : (oc + 1) * OC]
                            )
                            nc.vector.tensor_add(
                                y_t[:, oc * OC : (oc + 1) * OC],
                                t2,
                                biasb[:, oc * OC : (oc + 1) * OC],
                            )
                        nc.sync.dma_start(out=y_d[i * P : (i + 1) * P, :], in_=y_t)

    nc.compile()
    return nc


_NC_CACHE = {}


def _get_nc(n_rows, d, o, n_cores):
    key = (n_rows, d, o, n_cores)
    if key not in _NC_CACHE:
        _NC_CACHE[key] = build_nc(n_rows, d, o, n_cores)
    return _NC_CACHE[key]


def kernel(x: np.ndarray, weight: np.ndarray, bias: np.ndarray, **run_kwargs):
    b, n, d = x.shape
    o = weight.shape[0]
    rows = b * n
    n_rows = rows // N_CORES
    nc = _get_nc(n_rows, d, o, N_CORES)

    x_flat = np.ascontiguousarray(np.asarray(x, dtype=np.float32).reshape(rows, d))
    w = np.ascontiguousarray(np.asarray(weight, dtype=np.float32))
    bias = np.ascontiguousarray(np.asarray(bias, dtype=np.float32))

    in_maps = [
        {"x": x_flat[c * n_rows : (c + 1) * n_rows], "w": w, "b": bias}
        for c in range(N_CORES)
    ]
    res = run_bass_kernel_spmd(nc, in_maps, list(range(N_CORES)), **run_kwargs)
    y = np.concatenate([res.results[c]["y"] for c in range(N_CORES)], axis=0)
    out = y.reshape(b, n, o).astype(x.dtype, copy=False)
    if run_kwargs:
        return out, res
    return out


if __name__ == "__main__":
    x = np.random.randn(B, N, D).astype(np.float32)
    w = np.random.randn(O, D).astype(np.float32)
    bias = np.random.randn(O).astype(np.float32)
    y = kernel(x, w, bias)
    print(y.shape, y.dtype)
